# revision 7
# baseline (speedup 1.0000x reference)
"""Causal multi-head attention (B=4, S=2048, H=16, D=64, E=1024) on 8 TRN2 cores.

Sharding: data-parallel over batch (4) x tensor-parallel over heads (2 groups
of 8). Each core computes, for its (batch, head-group):
    q/k/v projections -> causal softmax attention -> output projection
and returns a partial [S, E] output (Wr row-split); the host adds the two
partials per batch.

All matmul operands are float32r (~TF32 precision at full PE rate).
Attention runs in the transposed layout (keys/head-dims on partitions) so no
on-chip transposes are needed; V carries an extra ones-column so the attn@V
matmul also emits the softmax denominators (output row 64).
"""

import numpy as np

import concourse.bacc as bacc
import concourse.bass as bass
import concourse.mybir as mybir
import concourse.tile as tile
from concourse.bass_utils import run_bass_kernel_spmd

HEADS = 16
HD = 64
EMB = 1024
B, S = 4, 2048
SCALE = 1.0 / 8.0
NCORES = 8
HPC = HEADS // 2          # heads per core (8)
GW = HPC * HD             # head-group width (512)

F32 = mybir.dt.float32
F32R = mybir.dt.float32r
EXP = mybir.ActivationFunctionType.Exp

NQC = 4                   # query chunks of 512
QW = 512                  # query chunk width
NKB = S // 128            # key blocks of 128 (16)
NEC = EMB // 128          # emb chunks (8)
NSB = S // 128            # seq blocks (16)


def build():
    nc = bacc.Bacc("TRN2", target_bir_lowering=False, debug=False)

    xt_d = nc.dram_tensor("xt", [EMB, S], F32R, kind="ExternalInput")
    # wq/wk pre-swizzled on host to [hp, p, e, n] so per-(hp) DMA is contiguous
    wq_d = nc.dram_tensor("wq", [4, 128, NEC, 128], F32R, kind="ExternalInput")
    wk_d = nc.dram_tensor("wk", [4, 128, NEC, 128], F32R, kind="ExternalInput")
    wv_d = nc.dram_tensor("wv", [EMB, GW], F32R, kind="ExternalInput")
    wr_d = nc.dram_tensor("wr", [GW, EMB], F32R, kind="ExternalInput")
    # consts: [:,0:128] causal tri mask, [:,128:256] ones, [:,256:640] zeros
    cn_d = nc.dram_tensor("consts", [128, 640], F32R, kind="ExternalInput")
    y_d = nc.dram_tensor("y", [S, EMB], F32, kind="ExternalOutput")

    with tile.TileContext(nc) as tc, nc.allow_low_precision(reason="f32r attn"):
        with (
            tc.tile_pool(name="persist", bufs=1) as pp,
            tc.tile_pool(name="qtp", bufs=1) as pq,
            tc.tile_pool(name="outp", bufs=1) as po,
            tc.tile_pool(name="attn", bufs=2) as pa,
            tc.tile_pool(name="recp", bufs=2) as prc,
            tc.tile_pool(name="bcp", bufs=1) as pbc,
            tc.tile_pool(name="ysb", bufs=1) as pyb,
            tc.tile_pool(name="ps_mm", bufs=1, space="PSUM") as ps_mm,
            tc.tile_pool(name="ps_y", bufs=1, space="PSUM") as ps_y,
            tc.tile_pool(name="ps_score", bufs=2, space="PSUM") as ps_sc,
            tc.tile_pool(name="ps_out", bufs=1, space="PSUM") as ps_out,
            tc.tile_pool(name="ps_bc", bufs=1, space="PSUM") as ps_bc,
        ):
            kt = pp.tile([128, NQC, S], F32R, tag="kt")
            v = pp.tile([128, NKB, HPC, HD + 1], F32R, tag="v")
            wr = pp.tile([128, 4, EMB], F32R, tag="wr")
            mo = pp.tile([128, 640], F32R, tag="consts")
            mask = mo[:, 0:128]
            ones = mo[0:1, 128:192]
            zeros = mo[:, 256:640]

            nc.sync.dma_start(mo[:], cn_d.ap())
            nc.sync.dma_start(wr[:], wr_d.ap().rearrange("(c p) n -> p c n", p=128))
            # ones column of v (softmax denominator trick)
            nc.sync.dma_start(v[:, :, :, HD], cn_d.ap()[:, 128:256])

            qtiles = {}

            with (
                tc.tile_pool(name="proj", bufs=1) as pj,
                tc.tile_pool(name="wql", bufs=1) as pwq,
                tc.tile_pool(name="wkl", bufs=1) as pwk,
            ):
                xt = pj.tile([128, NEC, S], F32R, tag="xt")
                wv = pj.tile([128, NEC, GW], F32R, tag="wv")

                for e in range(NEC):
                    nc.sync.dma_start(xt[:, e, :], xt_d.ap()[e * 128:(e + 1) * 128, :])
                nc.sync.dma_start(wv[:], wv_d.ap().rearrange("(c p) n -> p c n", p=128))

                # ---- P1: v = x @ Wv, natural layout [seq, head, 64] ----
                for sb in range(NSB):
                    ps = ps_mm.tile([128, GW], F32, tag="mm")
                    for e in range(NEC):
                        nc.tensor.matmul(
                            ps[:], xt[:, e, sb * 128:(sb + 1) * 128], wv[:, e, :],
                            start=(e == 0), stop=(e == NEC - 1),
                        )
                    nc.any.tensor_copy(
                        v[:, sb, :, 0:HD],
                        ps[:].rearrange("p (h d) -> p h d", d=HD),
                    )

                # ---- P2: qT (per query chunk) and kT head-pair tiles ----
                for c in range(NQC):
                    csl = slice(c * QW, (c + 1) * QW)
                    qtile = pq.tile([128, NQC, QW], F32R, tag="qt")
                    qtiles[c] = qtile
                    for dst, wsrc, pool in ((qtile, wq_d, pwq), (kt, wk_d, pwk)):
                        for hp in range(4):
                            wc = pool.tile([128, NEC, 128], F32R, tag="w")
                            nc.sync.dma_start(wc[:], wsrc.ap()[hp])
                            ps = ps_mm.tile([128, QW], F32, tag="mm")
                            for e in range(NEC):
                                nc.tensor.matmul(
                                    ps[:], wc[:, e, :], xt[:, e, csl],
                                    start=(e == 0), stop=(e == NEC - 1),
                                )
                            if dst is qtile:
                                nc.any.tensor_copy(qtile[:, hp, :], ps[:])
                            else:
                                nc.any.tensor_copy(kt[:, hp, csl], ps[:])

            # ---- P3/P4: attention + output projection per query chunk ----
            for qc in range(NQC):
                kbmax = 4 * (qc + 1)
                qtile = qtiles[qc]
                outtc = po.tile([128, NQC, QW], F32R, tag="outt")
                for h in range(HPC):
                    hp, ho = h // 2, (h % 2) * HD
                    out_ps = ps_out.tile([HD + 1, QW], F32, tag="out")
                    for g in range(kbmax // 2):
                        sc = ps_sc.tile([128, 2, QW], F32, tag="sc")
                        at = pa.tile([128, 2, QW], F32R, tag="at")
                        for s_ in range(2):
                            kb = 2 * g + s_
                            # scoresT block [keys, queries]
                            nc.tensor.matmul(
                                sc[:, s_, :],
                                kt[ho:ho + HD, hp, kb * 128:(kb + 1) * 128],
                                qtile[ho:ho + HD, hp, :],
                                start=True, stop=True,
                            )
                        nc.scalar.activation(at[:], sc[:], EXP)
                        for s_ in range(2):
                            kb = 2 * g + s_
                            j = kb - 4 * qc
                            if j >= 0:  # diagonal block: causal mask
                                if j > 0:
                                    nc.vector.tensor_copy(
                                        at[:, s_, 0:j * 128], zeros[:, 0:j * 128])
                                nc.vector.tensor_mul(
                                    at[:, s_, j * 128:(j + 1) * 128],
                                    at[:, s_, j * 128:(j + 1) * 128],
                                    mask,
                                )
                            nc.tensor.matmul(
                                out_ps[:],
                                v[:, kb, h, :],
                                at[:, s_, :],
                                start=(kb == 0), stop=(kb == kbmax - 1),
                            )
                    # rows 0..63 = (attn@v).T numerator, row 64 = denom
                    rec = prc.tile([1, QW], F32R, tag="rec")
                    nc.vector.reciprocal(rec[0:1, :], out_ps[HD:HD + 1, :])
                    bc_ps = ps_bc.tile([HD, QW], F32, tag="bcps")
                    nc.tensor.matmul(bc_ps[:], ones, rec[0:1, :],
                                     start=True, stop=True)
                    bc = pbc.tile([HD, QW], F32R, tag="bcs")
                    nc.any.tensor_copy(bc[:], bc_ps[:])
                    nc.vector.tensor_mul(
                        outtc[ho:ho + HD, hp, :], out_ps[0:HD, :], bc[:],
                    )

                # ---- P4: y rows for this query chunk ----
                for sbl in range(4):
                    sb = qc * 4 + sbl
                    ysb = pyb.tile([128, EMB], F32, tag="ysb")
                    for ncol in range(2):
                        ps = ps_y.tile([128, QW], F32, tag="ymm")
                        for hp in range(4):
                            nc.tensor.matmul(
                                ps[:],
                                outtc[:, hp, sbl * 128:(sbl + 1) * 128],
                                wr[:, hp, ncol * QW:(ncol + 1) * QW],
                                start=(hp == 0), stop=(hp == 3),
                            )
                        nc.any.tensor_copy(ysb[:, ncol * QW:(ncol + 1) * QW], ps[:])
                    nc.sync.dma_start(y_d.ap()[sb * 128:(sb + 1) * 128, :], ysb[:])

    nc.compile()
    return nc


_NC_CACHE = None


def _get_nc():
    global _NC_CACHE
    if _NC_CACHE is None:
        _NC_CACHE = build()
    return _NC_CACHE


def make_in_maps(x, Wq, Wk, Wv, Wr):
    x = np.ascontiguousarray(x, dtype=np.float32)
    Wq = np.asarray(Wq, dtype=np.float32)
    Wk = np.asarray(Wk, dtype=np.float32)
    Wv = np.asarray(Wv, dtype=np.float32)
    Wr = np.asarray(Wr, dtype=np.float32)

    consts = np.zeros((128, 640), dtype=np.float32)
    consts[:, 0:128] = np.triu(np.ones((128, 128), dtype=np.float32))
    consts[:, 128:256] = 1.0

    def swz(w):  # [1024, 512] -> [hp, p, e, n]
        return np.ascontiguousarray(
            w.reshape(NEC, 128, 4, 128).transpose(2, 1, 0, 3))

    in_maps = []
    for core in range(NCORES):
        b, g = divmod(core, 2)
        hs = slice(g * GW, (g + 1) * GW)
        in_maps.append({
            "xt": np.ascontiguousarray(x[b].T),
            "wq": swz(Wq[:, hs] * SCALE),
            "wk": swz(Wk[:, hs]),
            "wv": np.ascontiguousarray(Wv[:, hs]),
            "wr": np.ascontiguousarray(Wr[hs, :]),
            "consts": consts,
        })
    return in_maps


def kernel(x, Wq, Wk, Wv, Wr):
    in_maps = make_in_maps(x, Wq, Wk, Wv, Wr)
    nc = _get_nc()
    res = run_bass_kernel_spmd(nc, in_maps, core_ids=list(range(NCORES)))

    y = np.empty((B, S, EMB), dtype=np.float32)
    for b in range(B):
        y[b] = res.results[2 * b]["y"] + res.results[2 * b + 1]["y"]
    return y
